# revision 27
# baseline (speedup 1.0000x reference)
"""Trainium2 Bass kernel for nn_AttnPool (segment softmax attention pooling).

Reference computation:
    score = (h @ W + b)[:, 0]                      # [N]
    per-segment softmax of score over sorted segment ids `batch` (B segments)
    out[s, :] = sum_{n in seg s} softmax_weight(n) * h[n, :]    # [B, D]

Strategy (8 NeuronCores, SPMD):
  - batch is sorted, so assign whole segments to cores: core c owns segments
    [c*B/8, (c+1)*B/8).  No cross-core communication needed.
  - Host premultiplies hw = h * W (row-wise by feature) and packs to bf16.
    Then score = rowsum(hw), and the weighted feature sums are accumulated
    in hw-space; the final output is divided by W per feature on the host.
  - Softmax needs no max subtraction for this data (scores ~ N(0,1)), and
    softmax is shift invariant: out = (sum_n e_n * hw_n) / (sum_n e_n).
  - Per core, segments go to windows of WIN segments; nodes pack into
    128-row tiles that never straddle a window boundary (host pads).
    Windows are processed in groups of G for DMA efficiency.  Per group:
        scores = pairwise-add tree over the 128 hw columns  (DVE, 16-bit
                 2x mode; a plain tensor_reduce has no 2x mode on TRN2)
        e      = exp(score + b)                             (ACT, bf16)
    Per window:
        maskE[p, j, w] = e[p, j] if seg_rel[p, j] == w else 0
                 built by ONE Pool-engine local_scatter from host-computed
                 int16 indices (idx = j * WIN + seg_rel, -1 on padding;
                 the data slice starts on an even ev column because the
                 Q7 routine requires 4-byte-aligned data)
        psum  += maskE[:, j, :].T @ [1 | hw_j]    (PE bf16 matmul, accum)
    Two windows share one PSUM bank; their raw [WIN, 1+D] rows
    (denominator | numerator) are staged to SBUF by ACT and DMAd out; the
    host divides by the denominator and by W in one pass.
  - All cores run one shared program; per-(core,window) tile counts are
    padded to the max over cores (shared ragged schedule).
"""

import os
import sys

sys.path.insert(0, "/opt/trn_rl_repo")

import numpy as np

N_CORES = 8
D = 128
B_SEGS = 10000
N_NODES = 500000
WIN = int(os.environ.get("KWIN", "32"))  # segments per psum window
GRP = int(os.environ.get("KGRP", "6"))  # windows per DMA/score group
HDMA_SPLIT = int(os.environ.get("HDMA_SPLIT", "3"))
HB_BUFS = int(os.environ.get("HB_BUFS", "6"))
TREE_STOP = int(os.environ.get("TREE_STOP", "8"))  # tree width -> reduce

_CACHE: dict = {}
LAST_RESULTS = None


def _win_pad(tw: int) -> int:
    return tw + (tw & 1)


def _groups(n_windows):
    """Window group sizes: small groups at both ends to shorten the serial
    pipeline ramp (first matmul waits first group's DMA+tree+exp+scatter)
    and drain, big groups in the middle for DMA efficiency."""
    sizes = []
    rem = n_windows
    for s in (1, 1, 2):
        if rem > 2 * GRP:
            sizes.append(s)
            rem -= s
    tail = []
    for s in (1, 1, 2):
        if rem > 2 * GRP:
            tail.append(s)
            rem -= s
    while rem > 0:
        s = min(GRP, rem)
        sizes.append(s)
        rem -= s
    sizes += tail[::-1]
    bounds = []
    lo = 0
    for s in sizes:
        bounds.append((lo, lo + s))
        lo += s
    return bounds


def _sched(t_w, n_windows):
    """Per-window scatter layout: (goff, sh, nd, islot) per window.

    goff: tile offset within the window's DMA group; sh = goff&1 (the
    scatter data slice is shifted down one column to stay 4-byte aligned);
    nd: even number of data/idx columns; islot: column offset into idxt.
    """
    metas = []
    islot = 0
    for lo, hi in _groups(n_windows):
        goff = 0
        for w in range(lo, hi):
            tw = t_w[w]
            sh = goff & 1
            nd = _win_pad(sh + tw)
            metas.append((goff, sh, nd, islot))
            goff += tw
            islot += nd
    return metas, islot


def _build_program(t_w: tuple, n_windows: int, n_cores: int):
    import concourse.bacc as bacc
    import concourse.mybir as mybir
    import concourse.tile as tile

    f32 = mybir.dt.float32
    bf16 = mybir.dt.bfloat16
    f16 = mybir.dt.float16
    i16 = mybir.dt.int16
    alu = mybir.AluOpType
    act = mybir.ActivationFunctionType
    t_u = sum(t_w)
    metas, t_idx = _sched(t_w, n_windows)
    g_bounds = _groups(n_windows)

    nc = bacc.Bacc("TRN2", target_bir_lowering=False, debug=False,
                   num_devices=n_cores)
    hp = nc.dram_tensor("hp", [128, t_u, D + 1], bf16, kind="ExternalInput")
    idxt = nc.dram_tensor("idxt", [128, t_idx], i16, kind="ExternalInput")
    brep = nc.dram_tensor("brep", [128, 1], f32, kind="ExternalInput")
    out = nc.dram_tensor("out", [n_windows * WIN, D + 1], f32,
                         kind="ExternalOutput")

    with tile.TileContext(nc) as tc:
        with (
            tc.tile_pool(name="const", bufs=1) as cpool,
            tc.tile_pool(name="hbuf", bufs=HB_BUFS) as hpool,
            tc.tile_pool(name="tree", bufs=3) as tpool,
            tc.tile_pool(name="sc", bufs=4) as spool,
            tc.tile_pool(name="mask", bufs=10) as mpool,
            tc.tile_pool(name="psum", bufs=4, space="PSUM") as ppool,
            tc.tile_pool(name="outp", bufs=3) as opool,
        ):
            brep_sb = cpool.tile([128, 1], f32, tag="brep")
            nc.sync.dma_start(brep_sb[:], brep[:])
            idxt_sb = cpool.tile([128, t_idx], i16, tag="idxt")
            nc.sync.dma_start(idxt_sb[:], idxt[:])

            slot = 0
            for g_l, g_h in g_bounds:
                ws = list(range(g_l, g_h))
                gtw = sum(t_w[w] for w in ws)
                hb = hpool.tile([128, gtw, D + 1], bf16, tag="hb")
                nsp = min(HDMA_SPLIT, gtw)
                bounds = [i * gtw // nsp for i in range(nsp + 1)]
                for i in range(nsp):
                    lo, hi = bounds[i], bounds[i + 1]
                    if hi > lo:
                        nc.sync.dma_start(hb[:, lo:hi, :],
                                          hp[:, slot + lo:slot + hi, :])

                # score = rowsum over 128 hw columns: pairwise-add tree in
                # f16 down to TREE_STOP wide (tensor_tensor has a 2x 16-bit
                # mode; tensor_reduce does not), then one small reduce.
                with nc.allow_low_precision("f16 score tree accum"):
                    tprev = tpool.tile([128, gtw, 64], f16, tag="tL1")
                    nc.vector.tensor_tensor(
                        out=tprev[:], in0=hb[:, :, 1:65],
                        in1=hb[:, :, 65:129], op=alu.add)
                    width = 32
                    while width >= TREE_STOP:
                        tnext = tpool.tile([128, gtw, width], f16, tag="tLn")
                        nc.vector.tensor_tensor(
                            out=tnext[:], in0=tprev[:, :, 0:width],
                            in1=tprev[:, :, width:2 * width], op=alu.add)
                        tprev = tnext
                        width //= 2
                    # ev holds one slack column: scatter data slices may
                    # read one past the last tile (ignored via idx=-1)
                    sc = spool.tile([128, gtw + 1], f16, tag="sc")
                    nc.vector.tensor_reduce(
                        out=sc[:, 0:gtw],
                        in_=tprev[:], axis=mybir.AxisListType.X, op=alu.add)

                ev = spool.tile([128, gtw + 1], bf16, tag="ev")
                nc.scalar.activation(ev[:], sc[:], act.Exp,
                                     bias=brep_sb[:, 0:1], scale=1.0)

                # pairs of windows share one PSUM bank tile
                psp = None
                for wi, w in enumerate(ws):
                    tw = t_w[w]
                    if wi % 2 == 0:
                        psp = ppool.tile([WIN, 2, D + 1], f32, tag="psp")
                    ps = psp[:, wi % 2, :]
                    # local_scatter's data AP must start 4-byte aligned:
                    # shift down to an even ev column and pad nd to even.
                    # mkb is [tile, seg] so each matmul lhs is contiguous.
                    goff, sh, nd, islot = metas[w]
                    mkb = mpool.tile([128, nd, WIN], bf16, tag="mkb")
                    nc.gpsimd.local_scatter(
                        mkb.rearrange("p t w -> p (t w)"),
                        ev[:, goff - sh:goff - sh + nd],
                        idxt_sb[:, islot:islot + nd],
                        channels=128, num_elems=WIN * nd, num_idxs=nd)
                    for j in range(tw):
                        nc.tensor.matmul(ps, mkb[:, sh + j, :],
                                         hb[:, goff + j, :],
                                         start=(j == 0), stop=(j == tw - 1))

                    if wi % 2 == 1 or wi == len(ws) - 1:
                        npair = wi % 2 + 1
                        ot = opool.tile([WIN, 2, D + 1], f32, tag="ot")
                        nc.scalar.activation(ot[:, 0:npair, :],
                                             psp[:, 0:npair, :], act.Copy)
                        w0 = ws[wi - npair + 1]
                        dst = out[w0 * WIN:(w0 + npair) * WIN, :].rearrange(
                            "(t p) d -> p t d", p=WIN)
                        nc.scalar.dma_start(dst, ot[:, 0:npair, :])
                slot += gtw

    nc.compile()
    return nc


def _prep(h, batch, W, b, n_cores=N_CORES, b_segs=B_SEGS):
    import ml_dtypes

    bf16 = ml_dtypes.bfloat16
    h = np.ascontiguousarray(np.asarray(h, dtype=np.float32))
    batch = np.asarray(batch).astype(np.int64).ravel()
    w_vec = np.asarray(W, dtype=np.float32).reshape(-1)
    b_val = np.float32(np.asarray(b, dtype=np.float32).reshape(-1)[0])
    n, d = h.shape
    assert d == D and w_vec.shape[0] == D

    hw = h * w_vec[None, :]

    segc = b_segs // n_cores
    n_windows = (segc + WIN - 1) // WIN

    seg_bounds = []
    for c in range(n_cores):
        for w in range(n_windows):
            lo = c * segc + w * WIN
            hi = min(c * segc + (w + 1) * WIN, (c + 1) * segc)
            seg_bounds.append((lo, hi))
    seg_edges = np.array([sb[0] for sb in seg_bounds] + [b_segs],
                         dtype=np.int64)
    node_edges = np.searchsorted(batch, seg_edges, side="left")

    cnt = (node_edges[1:] - node_edges[:-1]).reshape(n_cores, n_windows)
    tiles = np.maximum((cnt + 127) // 128, 1)
    t_w = tuple(int(t) for t in tiles.max(axis=0))
    t_u = sum(t_w)
    metas, t_idx = _sched(t_w, n_windows)

    in_maps = []
    for c in range(n_cores):
        hp = np.zeros((t_u * 128, D + 1), dtype=np.float32)
        hp[:, 0] = 1.0
        idxr = np.full((128, t_idx), -1, dtype=np.int16)
        slot = 0
        for w in range(n_windows):
            k = c * n_windows + w
            tw = t_w[w]
            _goff, sh, nd, islot = metas[w]
            nlo, nhi = int(node_edges[k]), int(node_edges[k + 1])
            m = nhi - nlo
            if m > 0:
                hp[slot * 128:slot * 128 + m, 1:] = hw[nlo:nhi]
                seg_rel = (batch[nlo:nhi] - seg_bounds[k][0]).astype(
                    np.int64)
                # node r (global row slot*128+r) -> tile j = r//128,
                # partition p = r%128; scatter data column = sh + j;
                # mkb layout [tile, seg]: flat idx = (sh+j)*WIN + seg
                rr = np.arange(m)
                jj = rr // 128
                pp = rr % 128
                idxr[pp, islot + sh + jj] = (
                    (sh + jj) * WIN + seg_rel).astype(np.int16)
            slot += tw
        hp_t = np.ascontiguousarray(
            hp.reshape(t_u, 128, D + 1).transpose(1, 0, 2)).astype(bf16)
        in_maps.append({
            "hp": hp_t,
            "idxt": np.ascontiguousarray(idxr),
            "brep": np.full((128, 1), b_val, dtype=np.float32),
        })
    return in_maps, t_w, n_windows, segc


def _finish(core_outs, W, segc):
    w_vec = np.asarray(W, dtype=np.float32).reshape(-1)
    rows = np.concatenate([np.asarray(o[:segc], dtype=np.float32)
                           for o in core_outs], axis=0)
    den = np.maximum(rows[:, 0:1], 1e-30)
    return (rows[:, 1:] / den / w_vec[None, :]).astype(np.float32)


def _np_fallback(h, batch, W, b):
    h = np.asarray(h, dtype=np.float32)
    batch = np.asarray(batch).astype(np.int64).ravel()
    w_vec = np.asarray(W, dtype=np.float64).reshape(-1)
    b_val = float(np.asarray(b, dtype=np.float64).reshape(-1)[0])
    score = h.astype(np.float64) @ w_vec + b_val
    e = np.exp(score - score.max())
    den = np.zeros(B_SEGS)
    np.add.at(den, batch, e)
    num = np.zeros((B_SEGS, h.shape[1]))
    np.add.at(num, batch, e[:, None] * h.astype(np.float64))
    den = np.where(den > 0, den, 1.0)
    return (num / den[:, None]).astype(np.float32)


def kernel(h, batch, W, b):
    global LAST_RESULTS
    w_vec = np.asarray(W, dtype=np.float32).reshape(-1)
    if np.min(np.abs(w_vec)) < 1e-20:
        # hw-space accumulation cannot be unscaled for (near-)zero weights
        return _np_fallback(h, batch, W, b)

    from concourse.bass_utils import run_bass_kernel_spmd

    in_maps, t_w, n_windows, segc = _prep(h, batch, W, b)
    key = (t_w, n_windows, WIN, GRP, HDMA_SPLIT, HB_BUFS, TREE_STOP)
    if key not in _CACHE:
        _CACHE[key] = _build_program(t_w, n_windows, N_CORES)
    nc = _CACHE[key]

    res = run_bass_kernel_spmd(nc, in_maps, list(range(N_CORES)), trace=False)
    LAST_RESULTS = res
    return _finish([res.results[c]["out"] for c in range(N_CORES)], W, segc)


# revision 29
# speedup vs baseline: 1.0046x; 1.0046x over previous
"""Trainium2 Bass kernel for nn_AttnPool (segment softmax attention pooling).

Reference computation:
    score = (h @ W + b)[:, 0]                      # [N]
    per-segment softmax of score over sorted segment ids `batch` (B segments)
    out[s, :] = sum_{n in seg s} softmax_weight(n) * h[n, :]    # [B, D]

Strategy (8 NeuronCores, SPMD):
  - batch is sorted, so assign whole segments to cores: core c owns segments
    [c*B/8, (c+1)*B/8).  No cross-core communication needed.
  - Host premultiplies hw = h * W (row-wise by feature) and packs to bf16.
    Then score = rowsum(hw), and the weighted feature sums are accumulated
    in hw-space; the final output is divided by W per feature on the host.
  - Softmax needs no max subtraction for this data (scores ~ N(0,1)), and
    softmax is shift invariant: out = (sum_n e_n * hw_n) / (sum_n e_n).
  - Per core, segments go to windows of WIN segments; nodes pack into
    128-row tiles that never straddle a window boundary (host pads).
    Windows are processed in groups of G for DMA efficiency.  Per group:
        scores = pairwise-add tree over the 128 hw columns  (DVE, 16-bit
                 2x mode; a plain tensor_reduce has no 2x mode on TRN2)
        e      = exp(score + b)                             (ACT, bf16)
    Per window:
        maskE[p, j, w] = e[p, j] if seg_rel[p, j] == w else 0
                 built by ONE Pool-engine local_scatter from host-computed
                 int16 indices (idx = j * WIN + seg_rel, -1 on padding;
                 the data slice starts on an even ev column because the
                 Q7 routine requires 4-byte-aligned data)
        psum  += maskE[:, j, :].T @ [1 | hw_j]    (PE bf16 matmul, accum)
    Two windows share one PSUM bank; their raw [WIN, 1+D] rows
    (denominator | numerator) are staged to SBUF by ACT and DMAd out; the
    host divides by the denominator and by W in one pass.
  - All cores run one shared program; per-(core,window) tile counts are
    padded to the max over cores (shared ragged schedule).
"""

import os
import sys

sys.path.insert(0, "/opt/trn_rl_repo")

import numpy as np

N_CORES = 8
D = 128
B_SEGS = 10000
N_NODES = 500000
WIN = int(os.environ.get("KWIN", "32"))  # segments per psum window
GRP = int(os.environ.get("KGRP", "6"))  # windows per DMA/score group
HDMA_SPLIT = int(os.environ.get("HDMA_SPLIT", "3"))
HB_BUFS = int(os.environ.get("HB_BUFS", "6"))
TREE_STOP = int(os.environ.get("TREE_STOP", "8"))  # tree width -> reduce

_CACHE: dict = {}
LAST_RESULTS = None


def _win_pad(tw: int) -> int:
    return tw + (tw & 1)


def _groups(n_windows):
    """Window group sizes: small groups at both ends to shorten the serial
    pipeline ramp (first matmul waits first group's DMA+tree+exp+scatter)
    and drain, big groups in the middle for DMA efficiency."""
    sizes = []
    rem = n_windows
    for s in (1, 1, 2, 4):
        if rem > 2 * GRP:
            sizes.append(s)
            rem -= s
    tail = []
    for s in (1, 1, 2):
        if rem > 2 * GRP:
            tail.append(s)
            rem -= s
    while rem > 0:
        s = min(GRP, rem)
        sizes.append(s)
        rem -= s
    sizes += tail[::-1]
    bounds = []
    lo = 0
    for s in sizes:
        bounds.append((lo, lo + s))
        lo += s
    return bounds


def _sched(t_w, n_windows):
    """Per-window scatter layout: (goff, sh, nd, islot) per window.

    goff: tile offset within the window's DMA group; sh = goff&1 (the
    scatter data slice is shifted down one column to stay 4-byte aligned);
    nd: even number of data/idx columns; islot: column offset into idxt.
    """
    metas = []
    islot = 0
    for lo, hi in _groups(n_windows):
        goff = 0
        for w in range(lo, hi):
            tw = t_w[w]
            sh = goff & 1
            nd = _win_pad(sh + tw)
            metas.append((goff, sh, nd, islot))
            goff += tw
            islot += nd
    return metas, islot


def _build_program(t_w: tuple, n_windows: int, n_cores: int):
    import concourse.bacc as bacc
    import concourse.mybir as mybir
    import concourse.tile as tile

    f32 = mybir.dt.float32
    bf16 = mybir.dt.bfloat16
    f16 = mybir.dt.float16
    i16 = mybir.dt.int16
    alu = mybir.AluOpType
    act = mybir.ActivationFunctionType
    t_u = sum(t_w)
    metas, t_idx = _sched(t_w, n_windows)
    g_bounds = _groups(n_windows)

    nc = bacc.Bacc("TRN2", target_bir_lowering=False, debug=False,
                   num_devices=n_cores)
    hp = nc.dram_tensor("hp", [128, t_u, D + 1], bf16, kind="ExternalInput")
    idxt = nc.dram_tensor("idxt", [128, t_idx], i16, kind="ExternalInput")
    brep = nc.dram_tensor("brep", [128, 1], f32, kind="ExternalInput")
    out = nc.dram_tensor("out", [n_windows * WIN, D + 1], f32,
                         kind="ExternalOutput")

    with tile.TileContext(nc) as tc:
        with (
            tc.tile_pool(name="const", bufs=1) as cpool,
            tc.tile_pool(name="hbuf", bufs=HB_BUFS) as hpool,
            tc.tile_pool(name="tree", bufs=3) as tpool,
            tc.tile_pool(name="sc", bufs=4) as spool,
            tc.tile_pool(name="mask", bufs=10) as mpool,
            tc.tile_pool(name="psum", bufs=4, space="PSUM") as ppool,
            tc.tile_pool(name="outp", bufs=3) as opool,
        ):
            brep_sb = cpool.tile([128, 1], f32, tag="brep")
            nc.sync.dma_start(brep_sb[:], brep[:])
            idxt_sb = cpool.tile([128, t_idx], i16, tag="idxt")
            nc.sync.dma_start(idxt_sb[:], idxt[:])

            slot = 0
            for g_l, g_h in g_bounds:
                ws = list(range(g_l, g_h))
                gtw = sum(t_w[w] for w in ws)
                hb = hpool.tile([128, gtw, D + 1], bf16, tag="hb")
                nsp = min(HDMA_SPLIT, gtw)
                bounds = [i * gtw // nsp for i in range(nsp + 1)]
                for i in range(nsp):
                    lo, hi = bounds[i], bounds[i + 1]
                    if hi > lo:
                        nc.sync.dma_start(hb[:, lo:hi, :],
                                          hp[:, slot + lo:slot + hi, :])

                # score = rowsum over 128 hw columns: pairwise-add tree in
                # f16 down to TREE_STOP wide (tensor_tensor has a 2x 16-bit
                # mode; tensor_reduce does not), then one small reduce.
                # Level 1 is split along the DMA-split tile ranges so it
                # starts as each hb piece lands instead of after the whole
                # group's DMA.
                with nc.allow_low_precision("f16 score tree accum"):
                    tprev = tpool.tile([128, gtw, 64], f16, tag="tL1")
                    for i in range(nsp):
                        lo, hi = bounds[i], bounds[i + 1]
                        if hi > lo:
                            nc.vector.tensor_tensor(
                                out=tprev[:, lo:hi, :],
                                in0=hb[:, lo:hi, 1:65],
                                in1=hb[:, lo:hi, 65:129], op=alu.add)
                    width = 32
                    while width >= TREE_STOP:
                        tnext = tpool.tile([128, gtw, width], f16, tag="tLn")
                        nc.vector.tensor_tensor(
                            out=tnext[:], in0=tprev[:, :, 0:width],
                            in1=tprev[:, :, width:2 * width], op=alu.add)
                        tprev = tnext
                        width //= 2
                    # ev holds one slack column: scatter data slices may
                    # read one past the last tile (ignored via idx=-1)
                    sc = spool.tile([128, gtw + 1], f16, tag="sc")
                    nc.vector.tensor_reduce(
                        out=sc[:, 0:gtw],
                        in_=tprev[:], axis=mybir.AxisListType.X, op=alu.add)

                ev = spool.tile([128, gtw + 1], bf16, tag="ev")
                nc.scalar.activation(ev[:], sc[:], act.Exp,
                                     bias=brep_sb[:, 0:1], scale=1.0)

                # pairs of windows share one PSUM bank tile
                psp = None
                for wi, w in enumerate(ws):
                    tw = t_w[w]
                    if wi % 2 == 0:
                        psp = ppool.tile([WIN, 2, D + 1], f32, tag="psp")
                    ps = psp[:, wi % 2, :]
                    # local_scatter's data AP must start 4-byte aligned:
                    # shift down to an even ev column and pad nd to even.
                    # mkb is [tile, seg] so each matmul lhs is contiguous.
                    goff, sh, nd, islot = metas[w]
                    mkb = mpool.tile([128, nd, WIN], bf16, tag="mkb")
                    nc.gpsimd.local_scatter(
                        mkb.rearrange("p t w -> p (t w)"),
                        ev[:, goff - sh:goff - sh + nd],
                        idxt_sb[:, islot:islot + nd],
                        channels=128, num_elems=WIN * nd, num_idxs=nd)
                    for j in range(tw):
                        nc.tensor.matmul(ps, mkb[:, sh + j, :],
                                         hb[:, goff + j, :],
                                         start=(j == 0), stop=(j == tw - 1))

                    if wi % 2 == 1 or wi == len(ws) - 1:
                        npair = wi % 2 + 1
                        ot = opool.tile([WIN, 2, D + 1], f32, tag="ot")
                        nc.scalar.activation(ot[:, 0:npair, :],
                                             psp[:, 0:npair, :], act.Copy)
                        w0 = ws[wi - npair + 1]
                        dst = out[w0 * WIN:(w0 + npair) * WIN, :].rearrange(
                            "(t p) d -> p t d", p=WIN)
                        nc.scalar.dma_start(dst, ot[:, 0:npair, :])
                slot += gtw

    nc.compile()
    return nc


def _prep(h, batch, W, b, n_cores=N_CORES, b_segs=B_SEGS):
    import ml_dtypes

    bf16 = ml_dtypes.bfloat16
    h = np.ascontiguousarray(np.asarray(h, dtype=np.float32))
    batch = np.asarray(batch).astype(np.int64).ravel()
    w_vec = np.asarray(W, dtype=np.float32).reshape(-1)
    b_val = np.float32(np.asarray(b, dtype=np.float32).reshape(-1)[0])
    n, d = h.shape
    assert d == D and w_vec.shape[0] == D

    hw = h * w_vec[None, :]

    segc = b_segs // n_cores
    n_windows = (segc + WIN - 1) // WIN

    seg_bounds = []
    for c in range(n_cores):
        for w in range(n_windows):
            lo = c * segc + w * WIN
            hi = min(c * segc + (w + 1) * WIN, (c + 1) * segc)
            seg_bounds.append((lo, hi))
    seg_edges = np.array([sb[0] for sb in seg_bounds] + [b_segs],
                         dtype=np.int64)
    node_edges = np.searchsorted(batch, seg_edges, side="left")

    cnt = (node_edges[1:] - node_edges[:-1]).reshape(n_cores, n_windows)
    tiles = np.maximum((cnt + 127) // 128, 1)
    t_w = tuple(int(t) for t in tiles.max(axis=0))
    t_u = sum(t_w)
    metas, t_idx = _sched(t_w, n_windows)

    in_maps = []
    for c in range(n_cores):
        hp = np.zeros((t_u * 128, D + 1), dtype=np.float32)
        hp[:, 0] = 1.0
        idxr = np.full((128, t_idx), -1, dtype=np.int16)
        slot = 0
        for w in range(n_windows):
            k = c * n_windows + w
            tw = t_w[w]
            _goff, sh, nd, islot = metas[w]
            nlo, nhi = int(node_edges[k]), int(node_edges[k + 1])
            m = nhi - nlo
            if m > 0:
                hp[slot * 128:slot * 128 + m, 1:] = hw[nlo:nhi]
                seg_rel = (batch[nlo:nhi] - seg_bounds[k][0]).astype(
                    np.int64)
                # node r (global row slot*128+r) -> tile j = r//128,
                # partition p = r%128; scatter data column = sh + j;
                # mkb layout [tile, seg]: flat idx = (sh+j)*WIN + seg
                rr = np.arange(m)
                jj = rr // 128
                pp = rr % 128
                idxr[pp, islot + sh + jj] = (
                    (sh + jj) * WIN + seg_rel).astype(np.int16)
            slot += tw
        hp_t = np.ascontiguousarray(
            hp.reshape(t_u, 128, D + 1).transpose(1, 0, 2)).astype(bf16)
        in_maps.append({
            "hp": hp_t,
            "idxt": np.ascontiguousarray(idxr),
            "brep": np.full((128, 1), b_val, dtype=np.float32),
        })
    return in_maps, t_w, n_windows, segc


def _finish(core_outs, W, segc):
    w_vec = np.asarray(W, dtype=np.float32).reshape(-1)
    rows = np.concatenate([np.asarray(o[:segc], dtype=np.float32)
                           for o in core_outs], axis=0)
    den = np.maximum(rows[:, 0:1], 1e-30)
    return (rows[:, 1:] / den / w_vec[None, :]).astype(np.float32)


def _np_fallback(h, batch, W, b):
    h = np.asarray(h, dtype=np.float32)
    batch = np.asarray(batch).astype(np.int64).ravel()
    w_vec = np.asarray(W, dtype=np.float64).reshape(-1)
    b_val = float(np.asarray(b, dtype=np.float64).reshape(-1)[0])
    score = h.astype(np.float64) @ w_vec + b_val
    e = np.exp(score - score.max())
    den = np.zeros(B_SEGS)
    np.add.at(den, batch, e)
    num = np.zeros((B_SEGS, h.shape[1]))
    np.add.at(num, batch, e[:, None] * h.astype(np.float64))
    den = np.where(den > 0, den, 1.0)
    return (num / den[:, None]).astype(np.float32)


def kernel(h, batch, W, b):
    global LAST_RESULTS
    w_vec = np.asarray(W, dtype=np.float32).reshape(-1)
    if np.min(np.abs(w_vec)) < 1e-20:
        # hw-space accumulation cannot be unscaled for (near-)zero weights
        return _np_fallback(h, batch, W, b)

    from concourse.bass_utils import run_bass_kernel_spmd

    in_maps, t_w, n_windows, segc = _prep(h, batch, W, b)
    key = (t_w, n_windows, WIN, GRP, HDMA_SPLIT, HB_BUFS, TREE_STOP)
    if key not in _CACHE:
        _CACHE[key] = _build_program(t_w, n_windows, N_CORES)
    nc = _CACHE[key]

    res = run_bass_kernel_spmd(nc, in_maps, list(range(N_CORES)), trace=False)
    LAST_RESULTS = res
    return _finish([res.results[c]["out"] for c in range(N_CORES)], W, segc)


# revision 30
# speedup vs baseline: 1.0426x; 1.0378x over previous
"""Trainium2 Bass kernel for nn_AttnPool (segment softmax attention pooling).

Reference computation:
    score = (h @ W + b)[:, 0]                      # [N]
    per-segment softmax of score over sorted segment ids `batch` (B segments)
    out[s, :] = sum_{n in seg s} softmax_weight(n) * h[n, :]    # [B, D]

Strategy (8 NeuronCores, SPMD):
  - batch is sorted, so assign whole segments to cores: core c owns segments
    [c*B/8, (c+1)*B/8).  No cross-core communication needed.
  - Host premultiplies hw = h * W (row-wise by feature) and packs to bf16.
    Then score = rowsum(hw), and the weighted feature sums are accumulated
    in hw-space; the final output is divided by W per feature on the host.
  - Softmax needs no max subtraction for this data (scores ~ N(0,1)), and
    softmax is shift invariant: out = (sum_n e_n * hw_n) / (sum_n e_n).
  - Per core, segments go to windows of WIN segments; nodes pack into
    128-row tiles that never straddle a window boundary (host pads).
    Windows are processed in groups of G for DMA efficiency.  Per group:
        scores = pairwise-add tree over the 128 hw columns  (DVE, 16-bit
                 2x mode; a plain tensor_reduce has no 2x mode on TRN2)
        e      = exp(score + b)                             (ACT, bf16)
    Per window:
        maskE[p, j, w] = e[p, j] if seg_rel[p, j] == w else 0
                 built by ONE Pool-engine local_scatter from host-computed
                 int16 indices (idx = j * WIN + seg_rel, -1 on padding;
                 the data slice starts on an even ev column because the
                 Q7 routine requires 4-byte-aligned data)
        psum  += maskE[:, j, :].T @ [1 | hw_j]    (PE bf16 matmul, accum)
    Two windows share one PSUM bank; their raw [WIN, 1+D] rows
    (denominator | numerator) are staged to SBUF by ACT and DMAd out; the
    host divides by the denominator and by W in one pass.
  - All cores run one shared program; per-(core,window) tile counts are
    padded to the max over cores (shared ragged schedule).
"""

import os
import sys

sys.path.insert(0, "/opt/trn_rl_repo")

import numpy as np

N_CORES = 8
D = 128
B_SEGS = 10000
N_NODES = 500000
WIN = int(os.environ.get("KWIN", "32"))  # segments per psum window
GRP = int(os.environ.get("KGRP", "6"))  # windows per DMA/score group
HDMA_SPLIT = int(os.environ.get("HDMA_SPLIT", "3"))
HB_BUFS = int(os.environ.get("HB_BUFS", "6"))
TREE_STOP = int(os.environ.get("TREE_STOP", "8"))  # tree width -> reduce

_CACHE: dict = {}
LAST_RESULTS = None


def _win_pad(tw: int) -> int:
    return tw + (tw & 1)


def _groups(n_windows):
    """Window group sizes: small groups at both ends to shorten the serial
    pipeline ramp (first matmul waits first group's DMA+tree+exp+scatter)
    and drain, big groups in the middle for DMA efficiency."""
    sizes = []
    rem = n_windows
    for s in (1, 1, 2, 4):
        if rem > 2 * GRP:
            sizes.append(s)
            rem -= s
    tail = []
    for s in (1, 1, 2):
        if rem > 2 * GRP:
            tail.append(s)
            rem -= s
    while rem > 0:
        s = min(GRP, rem)
        sizes.append(s)
        rem -= s
    sizes += tail[::-1]
    bounds = []
    lo = 0
    for s in sizes:
        bounds.append((lo, lo + s))
        lo += s
    return bounds


def _sched(t_w, n_windows):
    """Per-window scatter layout: (goff, sh, nd, islot) per window.

    goff: tile offset within the window's DMA group; sh = goff&1 (the
    scatter data slice is shifted down one column to stay 4-byte aligned);
    nd: even number of data/idx columns; islot: column offset into idxt.
    """
    metas = []
    islot = 0
    for lo, hi in _groups(n_windows):
        goff = 0
        for w in range(lo, hi):
            tw = t_w[w]
            sh = goff & 1
            nd = _win_pad(sh + tw)
            metas.append((goff, sh, nd, islot))
            goff += tw
            islot += nd
    return metas, islot


def _build_program(t_w: tuple, n_windows: int, n_cores: int):
    import concourse.bacc as bacc
    import concourse.mybir as mybir
    import concourse.tile as tile

    f32 = mybir.dt.float32
    bf16 = mybir.dt.bfloat16
    f16 = mybir.dt.float16
    i16 = mybir.dt.int16
    alu = mybir.AluOpType
    act = mybir.ActivationFunctionType
    t_u = sum(t_w)
    metas, t_idx = _sched(t_w, n_windows)
    g_bounds = _groups(n_windows)

    nc = bacc.Bacc("TRN2", target_bir_lowering=False, debug=False,
                   num_devices=n_cores)
    hp = nc.dram_tensor("hp", [128, t_u, D + 1], bf16, kind="ExternalInput")
    idxt = nc.dram_tensor("idxt", [128, t_idx], i16, kind="ExternalInput")
    brep = nc.dram_tensor("brep", [128, 1], f32, kind="ExternalInput")
    out = nc.dram_tensor("out", [n_windows * WIN, D + 1], f32,
                         kind="ExternalOutput")

    with tile.TileContext(nc) as tc:
        with (
            tc.tile_pool(name="const", bufs=1) as cpool,
            tc.tile_pool(name="hbuf", bufs=HB_BUFS) as hpool,
            tc.tile_pool(name="tree", bufs=3) as tpool,
            tc.tile_pool(name="sc", bufs=4) as spool,
            tc.tile_pool(name="mask", bufs=10) as mpool,
            tc.tile_pool(name="psum", bufs=4, space="PSUM") as ppool,
            tc.tile_pool(name="outp", bufs=3) as opool,
        ):
            brep_sb = cpool.tile([128, 1], f32, tag="brep")
            nc.sync.dma_start(brep_sb[:], brep[:])
            idxt_sb = cpool.tile([128, t_idx], i16, tag="idxt")
            nc.sync.dma_start(idxt_sb[:], idxt[:])

            slot = 0
            for g_l, g_h in g_bounds:
                ws = list(range(g_l, g_h))
                gtw = sum(t_w[w] for w in ws)
                hb = hpool.tile([128, gtw, D + 1], bf16, tag="hb")
                nsp = min(HDMA_SPLIT, gtw)
                bounds = [i * gtw // nsp for i in range(nsp + 1)]
                for i in range(nsp):
                    lo, hi = bounds[i], bounds[i + 1]
                    if hi > lo:
                        # alternate dispatch queues: the serial ~1.3us
                        # per-dma_start dispatch cost on one engine
                        # otherwise paces the whole prefetch pipeline
                        eng = nc.sync if i % 2 == 0 else nc.gpsimd
                        eng.dma_start(hb[:, lo:hi, :],
                                      hp[:, slot + lo:slot + hi, :])

                # score = rowsum over 128 hw columns: pairwise-add tree in
                # f16 down to TREE_STOP wide (tensor_tensor has a 2x 16-bit
                # mode; tensor_reduce does not), then one small reduce.
                # Level 1 is split along the DMA-split tile ranges so it
                # starts as each hb piece lands instead of after the whole
                # group's DMA.
                with nc.allow_low_precision("f16 score tree accum"):
                    tprev = tpool.tile([128, gtw, 64], f16, tag="tL1")
                    for i in range(nsp):
                        lo, hi = bounds[i], bounds[i + 1]
                        if hi > lo:
                            nc.vector.tensor_tensor(
                                out=tprev[:, lo:hi, :],
                                in0=hb[:, lo:hi, 1:65],
                                in1=hb[:, lo:hi, 65:129], op=alu.add)
                    width = 32
                    while width >= TREE_STOP:
                        tnext = tpool.tile([128, gtw, width], f16, tag="tLn")
                        nc.vector.tensor_tensor(
                            out=tnext[:], in0=tprev[:, :, 0:width],
                            in1=tprev[:, :, width:2 * width], op=alu.add)
                        tprev = tnext
                        width //= 2
                    # ev holds one slack column: scatter data slices may
                    # read one past the last tile (ignored via idx=-1)
                    sc = spool.tile([128, gtw + 1], f16, tag="sc")
                    nc.vector.tensor_reduce(
                        out=sc[:, 0:gtw],
                        in_=tprev[:], axis=mybir.AxisListType.X, op=alu.add)

                ev = spool.tile([128, gtw + 1], bf16, tag="ev")
                nc.scalar.activation(ev[:], sc[:], act.Exp,
                                     bias=brep_sb[:, 0:1], scale=1.0)

                # pairs of windows share one PSUM bank tile
                psp = None
                for wi, w in enumerate(ws):
                    tw = t_w[w]
                    if wi % 2 == 0:
                        psp = ppool.tile([WIN, 2, D + 1], f32, tag="psp")
                    ps = psp[:, wi % 2, :]
                    # local_scatter's data AP must start 4-byte aligned:
                    # shift down to an even ev column and pad nd to even.
                    # mkb is [tile, seg] so each matmul lhs is contiguous.
                    goff, sh, nd, islot = metas[w]
                    mkb = mpool.tile([128, nd, WIN], bf16, tag="mkb")
                    nc.gpsimd.local_scatter(
                        mkb.rearrange("p t w -> p (t w)"),
                        ev[:, goff - sh:goff - sh + nd],
                        idxt_sb[:, islot:islot + nd],
                        channels=128, num_elems=WIN * nd, num_idxs=nd)
                    for j in range(tw):
                        nc.tensor.matmul(ps, mkb[:, sh + j, :],
                                         hb[:, goff + j, :],
                                         start=(j == 0), stop=(j == tw - 1))

                    if wi % 2 == 1 or wi == len(ws) - 1:
                        npair = wi % 2 + 1
                        ot = opool.tile([WIN, 2, D + 1], f32, tag="ot")
                        nc.scalar.activation(ot[:, 0:npair, :],
                                             psp[:, 0:npair, :], act.Copy)
                        w0 = ws[wi - npair + 1]
                        dst = out[w0 * WIN:(w0 + npair) * WIN, :].rearrange(
                            "(t p) d -> p t d", p=WIN)
                        nc.scalar.dma_start(dst, ot[:, 0:npair, :])
                slot += gtw

    nc.compile()
    return nc


def _prep(h, batch, W, b, n_cores=N_CORES, b_segs=B_SEGS):
    import ml_dtypes

    bf16 = ml_dtypes.bfloat16
    h = np.ascontiguousarray(np.asarray(h, dtype=np.float32))
    batch = np.asarray(batch).astype(np.int64).ravel()
    w_vec = np.asarray(W, dtype=np.float32).reshape(-1)
    b_val = np.float32(np.asarray(b, dtype=np.float32).reshape(-1)[0])
    n, d = h.shape
    assert d == D and w_vec.shape[0] == D

    hw = h * w_vec[None, :]

    segc = b_segs // n_cores
    n_windows = (segc + WIN - 1) // WIN

    seg_bounds = []
    for c in range(n_cores):
        for w in range(n_windows):
            lo = c * segc + w * WIN
            hi = min(c * segc + (w + 1) * WIN, (c + 1) * segc)
            seg_bounds.append((lo, hi))
    seg_edges = np.array([sb[0] for sb in seg_bounds] + [b_segs],
                         dtype=np.int64)
    node_edges = np.searchsorted(batch, seg_edges, side="left")

    cnt = (node_edges[1:] - node_edges[:-1]).reshape(n_cores, n_windows)
    tiles = np.maximum((cnt + 127) // 128, 1)
    t_w = tuple(int(t) for t in tiles.max(axis=0))
    t_u = sum(t_w)
    metas, t_idx = _sched(t_w, n_windows)

    in_maps = []
    for c in range(n_cores):
        hp = np.zeros((t_u * 128, D + 1), dtype=np.float32)
        hp[:, 0] = 1.0
        idxr = np.full((128, t_idx), -1, dtype=np.int16)
        slot = 0
        for w in range(n_windows):
            k = c * n_windows + w
            tw = t_w[w]
            _goff, sh, nd, islot = metas[w]
            nlo, nhi = int(node_edges[k]), int(node_edges[k + 1])
            m = nhi - nlo
            if m > 0:
                hp[slot * 128:slot * 128 + m, 1:] = hw[nlo:nhi]
                seg_rel = (batch[nlo:nhi] - seg_bounds[k][0]).astype(
                    np.int64)
                # node r (global row slot*128+r) -> tile j = r//128,
                # partition p = r%128; scatter data column = sh + j;
                # mkb layout [tile, seg]: flat idx = (sh+j)*WIN + seg
                rr = np.arange(m)
                jj = rr // 128
                pp = rr % 128
                idxr[pp, islot + sh + jj] = (
                    (sh + jj) * WIN + seg_rel).astype(np.int16)
            slot += tw
        hp_t = np.ascontiguousarray(
            hp.reshape(t_u, 128, D + 1).transpose(1, 0, 2)).astype(bf16)
        in_maps.append({
            "hp": hp_t,
            "idxt": np.ascontiguousarray(idxr),
            "brep": np.full((128, 1), b_val, dtype=np.float32),
        })
    return in_maps, t_w, n_windows, segc


def _finish(core_outs, W, segc):
    w_vec = np.asarray(W, dtype=np.float32).reshape(-1)
    rows = np.concatenate([np.asarray(o[:segc], dtype=np.float32)
                           for o in core_outs], axis=0)
    den = np.maximum(rows[:, 0:1], 1e-30)
    return (rows[:, 1:] / den / w_vec[None, :]).astype(np.float32)


def _np_fallback(h, batch, W, b):
    h = np.asarray(h, dtype=np.float32)
    batch = np.asarray(batch).astype(np.int64).ravel()
    w_vec = np.asarray(W, dtype=np.float64).reshape(-1)
    b_val = float(np.asarray(b, dtype=np.float64).reshape(-1)[0])
    score = h.astype(np.float64) @ w_vec + b_val
    e = np.exp(score - score.max())
    den = np.zeros(B_SEGS)
    np.add.at(den, batch, e)
    num = np.zeros((B_SEGS, h.shape[1]))
    np.add.at(num, batch, e[:, None] * h.astype(np.float64))
    den = np.where(den > 0, den, 1.0)
    return (num / den[:, None]).astype(np.float32)


def kernel(h, batch, W, b):
    global LAST_RESULTS
    w_vec = np.asarray(W, dtype=np.float32).reshape(-1)
    if np.min(np.abs(w_vec)) < 1e-20:
        # hw-space accumulation cannot be unscaled for (near-)zero weights
        return _np_fallback(h, batch, W, b)

    from concourse.bass_utils import run_bass_kernel_spmd

    in_maps, t_w, n_windows, segc = _prep(h, batch, W, b)
    key = (t_w, n_windows, WIN, GRP, HDMA_SPLIT, HB_BUFS, TREE_STOP)
    if key not in _CACHE:
        _CACHE[key] = _build_program(t_w, n_windows, N_CORES)
    nc = _CACHE[key]

    res = run_bass_kernel_spmd(nc, in_maps, list(range(N_CORES)), trace=False)
    LAST_RESULTS = res
    return _finish([res.results[c]["out"] for c in range(N_CORES)], W, segc)


# revision 31
# speedup vs baseline: 1.1599x; 1.1125x over previous
"""Trainium2 Bass kernel for nn_AttnPool (segment softmax attention pooling).

Reference computation:
    score = (h @ W + b)[:, 0]                      # [N]
    per-segment softmax of score over sorted segment ids `batch` (B segments)
    out[s, :] = sum_{n in seg s} softmax_weight(n) * h[n, :]    # [B, D]

Strategy (8 NeuronCores, SPMD):
  - batch is sorted, so assign whole segments to cores: core c owns segments
    [c*B/8, (c+1)*B/8).  No cross-core communication needed.
  - Host premultiplies hw = h * W (row-wise by feature) and packs to bf16.
    Then score = rowsum(hw), and the weighted feature sums are accumulated
    in hw-space; the final output is divided by W per feature on the host.
  - Softmax needs no max subtraction for this data (scores ~ N(0,1)), and
    softmax is shift invariant: out = (sum_n e_n * hw_n) / (sum_n e_n).
  - Per core, segments go to windows of WIN segments; nodes pack into
    128-row tiles that never straddle a window boundary (host pads).
    Windows are processed in groups of G for DMA efficiency.  Per group:
        scores = pairwise-add tree over the 128 hw columns  (DVE, 16-bit
                 2x mode; a plain tensor_reduce has no 2x mode on TRN2)
        e      = exp(score + b)                             (ACT, bf16)
    Per window:
        maskE[p, j, w] = e[p, j] if seg_rel[p, j] == w else 0
                 built by ONE Pool-engine local_scatter from host-computed
                 int16 indices (idx = j * WIN + seg_rel, -1 on padding;
                 the data slice starts on an even ev column because the
                 Q7 routine requires 4-byte-aligned data)
        psum  += maskE[:, j, :].T @ [1 | hw_j]    (PE bf16 matmul, accum)
    Two windows share one PSUM bank; their raw [WIN, 1+D] rows
    (denominator | numerator) are staged to SBUF by ACT and DMAd out; the
    host divides by the denominator and by W in one pass.
  - All cores run one shared program; per-(core,window) tile counts are
    padded to the max over cores (shared ragged schedule).
"""

import os
import sys

sys.path.insert(0, "/opt/trn_rl_repo")

import numpy as np

N_CORES = 8
D = 128
B_SEGS = 10000
N_NODES = 500000
WIN = int(os.environ.get("KWIN", "32"))  # segments per psum window
GRP = int(os.environ.get("KGRP", "6"))  # windows per DMA/score group
HDMA_SPLIT = int(os.environ.get("HDMA_SPLIT", "3"))
HB_BUFS = int(os.environ.get("HB_BUFS", "6"))
TREE_STOP = int(os.environ.get("TREE_STOP", "8"))  # tree width -> reduce

_CACHE: dict = {}
LAST_RESULTS = None


def _win_pad(tw: int) -> int:
    return tw + (tw & 1)


def _groups(n_windows):
    """Window group sizes: small groups at both ends to shorten the serial
    pipeline ramp (first matmul waits first group's DMA+tree+exp+scatter)
    and drain, big groups in the middle for DMA efficiency."""
    sizes = []
    rem = n_windows
    for s in (1, 1, 2, 4):
        if rem > 2 * GRP:
            sizes.append(s)
            rem -= s
    tail = []
    for s in (1, 1, 2):
        if rem > 2 * GRP:
            tail.append(s)
            rem -= s
    while rem > 0:
        s = min(GRP, rem)
        sizes.append(s)
        rem -= s
    sizes += tail[::-1]
    bounds = []
    lo = 0
    for s in sizes:
        bounds.append((lo, lo + s))
        lo += s
    return bounds


def _sched(t_w, n_windows):
    """Per-window scatter layout: (goff, sh, nd, islot) per window.

    goff: tile offset within the window's DMA group; sh = goff&1 (the
    scatter data slice is shifted down one column to stay 4-byte aligned);
    nd: even number of data/idx columns; islot: column offset into idxt.
    """
    metas = []
    islot = 0
    for lo, hi in _groups(n_windows):
        goff = 0
        for w in range(lo, hi):
            tw = t_w[w]
            sh = goff & 1
            nd = _win_pad(sh + tw)
            metas.append((goff, sh, nd, islot))
            goff += tw
            islot += nd
    return metas, islot


def _build_program(t_w: tuple, n_windows: int, n_cores: int):
    import concourse.bacc as bacc
    import concourse.mybir as mybir
    import concourse.tile as tile

    f32 = mybir.dt.float32
    bf16 = mybir.dt.bfloat16
    f16 = mybir.dt.float16
    i16 = mybir.dt.int16
    alu = mybir.AluOpType
    act = mybir.ActivationFunctionType
    t_u = sum(t_w)
    metas, t_idx = _sched(t_w, n_windows)
    g_bounds = _groups(n_windows)

    nc = bacc.Bacc("TRN2", target_bir_lowering=False, debug=False,
                   num_devices=n_cores)
    hp = nc.dram_tensor("hp", [128, t_u, D + 1], bf16, kind="ExternalInput")
    idxt = nc.dram_tensor("idxt", [128, t_idx], i16, kind="ExternalInput")
    brep = nc.dram_tensor("brep", [128, 1], f32, kind="ExternalInput")
    out = nc.dram_tensor("out", [n_windows * WIN, D + 1], f32,
                         kind="ExternalOutput")

    with tile.TileContext(nc) as tc:
        with (
            tc.tile_pool(name="const", bufs=1) as cpool,
            tc.tile_pool(name="hbuf", bufs=HB_BUFS) as hpool,
            tc.tile_pool(name="tree", bufs=3) as tpool,
            tc.tile_pool(name="sc", bufs=4) as spool,
            tc.tile_pool(name="mask", bufs=10) as mpool,
            tc.tile_pool(name="psum", bufs=4, space="PSUM") as ppool,
            tc.tile_pool(name="outp", bufs=3) as opool,
        ):
            brep_sb = cpool.tile([128, 1], f32, tag="brep")
            nc.sync.dma_start(brep_sb[:], brep[:])
            idxt_sb = cpool.tile([128, t_idx], i16, tag="idxt")
            nc.sync.dma_start(idxt_sb[:], idxt[:])

            slot = 0
            for g_l, g_h in g_bounds:
                ws = list(range(g_l, g_h))
                gtw = sum(t_w[w] for w in ws)
                hb = hpool.tile([128, gtw, D + 1], bf16, tag="hb")
                nsp = min(HDMA_SPLIT, gtw)
                bounds = [i * gtw // nsp for i in range(nsp + 1)]
                for i in range(nsp):
                    lo, hi = bounds[i], bounds[i + 1]
                    if hi > lo:
                        nc.sync.dma_start(hb[:, lo:hi, :],
                                          hp[:, slot + lo:slot + hi, :])

                # score = rowsum over 128 hw columns: pairwise-add tree in
                # f16 down to TREE_STOP wide (tensor_tensor has a 2x 16-bit
                # mode; tensor_reduce does not), then one small reduce.
                # Level 1 is split along the DMA-split tile ranges so it
                # starts as each hb piece lands instead of after the whole
                # group's DMA.
                with nc.allow_low_precision("f16 score tree accum"):
                    tprev = tpool.tile([128, gtw, 64], f16, tag="tL1")
                    for i in range(nsp):
                        lo, hi = bounds[i], bounds[i + 1]
                        if hi > lo:
                            nc.vector.tensor_tensor(
                                out=tprev[:, lo:hi, :],
                                in0=hb[:, lo:hi, 1:65],
                                in1=hb[:, lo:hi, 65:129], op=alu.add)
                    width = 32
                    while width >= TREE_STOP:
                        tnext = tpool.tile([128, gtw, width], f16, tag="tLn")
                        nc.vector.tensor_tensor(
                            out=tnext[:], in0=tprev[:, :, 0:width],
                            in1=tprev[:, :, width:2 * width], op=alu.add)
                        tprev = tnext
                        width //= 2
                    # ev holds one slack column: scatter data slices may
                    # read one past the last tile (ignored via idx=-1)
                    sc = spool.tile([128, gtw + 1], f16, tag="sc")
                    nc.vector.tensor_reduce(
                        out=sc[:, 0:gtw],
                        in_=tprev[:], axis=mybir.AxisListType.X, op=alu.add)

                ev = spool.tile([128, gtw + 1], bf16, tag="ev")
                nc.scalar.activation(ev[:], sc[:], act.Exp,
                                     bias=brep_sb[:, 0:1], scale=1.0)

                # pairs of windows share one PSUM bank tile
                psp = None
                for wi, w in enumerate(ws):
                    tw = t_w[w]
                    if wi % 2 == 0:
                        psp = ppool.tile([WIN, 2, D + 1], f32, tag="psp")
                    ps = psp[:, wi % 2, :]
                    # local_scatter's data AP must start 4-byte aligned:
                    # shift down to an even ev column and pad nd to even.
                    # mkb is [tile, seg] so each matmul lhs is contiguous.
                    goff, sh, nd, islot = metas[w]
                    mkb = mpool.tile([128, nd, WIN], bf16, tag="mkb")
                    nc.gpsimd.local_scatter(
                        mkb.rearrange("p t w -> p (t w)"),
                        ev[:, goff - sh:goff - sh + nd],
                        idxt_sb[:, islot:islot + nd],
                        channels=128, num_elems=WIN * nd, num_idxs=nd)
                    for j in range(tw):
                        nc.tensor.matmul(ps, mkb[:, sh + j, :],
                                         hb[:, goff + j, :],
                                         start=(j == 0), stop=(j == tw - 1))

                    if wi % 2 == 1 or wi == len(ws) - 1:
                        npair = wi % 2 + 1
                        ot = opool.tile([WIN, 2, D + 1], f32, tag="ot")
                        nc.scalar.activation(ot[:, 0:npair, :],
                                             psp[:, 0:npair, :], act.Copy)
                        w0 = ws[wi - npair + 1]
                        dst = out[w0 * WIN:(w0 + npair) * WIN, :].rearrange(
                            "(t p) d -> p t d", p=WIN)
                        nc.scalar.dma_start(dst, ot[:, 0:npair, :])
                slot += gtw

    nc.compile()
    return nc


def _prep(h, batch, W, b, n_cores=N_CORES, b_segs=B_SEGS):
    import ml_dtypes

    bf16 = ml_dtypes.bfloat16
    h = np.ascontiguousarray(np.asarray(h, dtype=np.float32))
    batch = np.asarray(batch).astype(np.int64).ravel()
    w_vec = np.asarray(W, dtype=np.float32).reshape(-1)
    b_val = np.float32(np.asarray(b, dtype=np.float32).reshape(-1)[0])
    n, d = h.shape
    assert d == D and w_vec.shape[0] == D

    hw = h * w_vec[None, :]

    segc = b_segs // n_cores
    n_windows = (segc + WIN - 1) // WIN

    seg_bounds = []
    for c in range(n_cores):
        for w in range(n_windows):
            lo = c * segc + w * WIN
            hi = min(c * segc + (w + 1) * WIN, (c + 1) * segc)
            seg_bounds.append((lo, hi))
    seg_edges = np.array([sb[0] for sb in seg_bounds] + [b_segs],
                         dtype=np.int64)
    node_edges = np.searchsorted(batch, seg_edges, side="left")

    cnt = (node_edges[1:] - node_edges[:-1]).reshape(n_cores, n_windows)
    tiles = np.maximum((cnt + 127) // 128, 1)
    t_w = tuple(int(t) for t in tiles.max(axis=0))
    t_u = sum(t_w)
    metas, t_idx = _sched(t_w, n_windows)

    in_maps = []
    for c in range(n_cores):
        hp = np.zeros((t_u * 128, D + 1), dtype=np.float32)
        hp[:, 0] = 1.0
        idxr = np.full((128, t_idx), -1, dtype=np.int16)
        slot = 0
        for w in range(n_windows):
            k = c * n_windows + w
            tw = t_w[w]
            _goff, sh, nd, islot = metas[w]
            nlo, nhi = int(node_edges[k]), int(node_edges[k + 1])
            m = nhi - nlo
            if m > 0:
                hp[slot * 128:slot * 128 + m, 1:] = hw[nlo:nhi]
                seg_rel = (batch[nlo:nhi] - seg_bounds[k][0]).astype(
                    np.int64)
                # node r (global row slot*128+r) -> tile j = r//128,
                # partition p = r%128; scatter data column = sh + j;
                # mkb layout [tile, seg]: flat idx = (sh+j)*WIN + seg
                rr = np.arange(m)
                jj = rr // 128
                pp = rr % 128
                idxr[pp, islot + sh + jj] = (
                    (sh + jj) * WIN + seg_rel).astype(np.int16)
            slot += tw
        hp_t = np.ascontiguousarray(
            hp.reshape(t_u, 128, D + 1).transpose(1, 0, 2)).astype(bf16)
        in_maps.append({
            "hp": hp_t,
            "idxt": np.ascontiguousarray(idxr),
            "brep": np.full((128, 1), b_val, dtype=np.float32),
        })
    return in_maps, t_w, n_windows, segc


def _finish(core_outs, W, segc):
    w_vec = np.asarray(W, dtype=np.float32).reshape(-1)
    rows = np.concatenate([np.asarray(o[:segc], dtype=np.float32)
                           for o in core_outs], axis=0)
    den = np.maximum(rows[:, 0:1], 1e-30)
    return (rows[:, 1:] / den / w_vec[None, :]).astype(np.float32)


def _np_fallback(h, batch, W, b):
    h = np.asarray(h, dtype=np.float32)
    batch = np.asarray(batch).astype(np.int64).ravel()
    w_vec = np.asarray(W, dtype=np.float64).reshape(-1)
    b_val = float(np.asarray(b, dtype=np.float64).reshape(-1)[0])
    score = h.astype(np.float64) @ w_vec + b_val
    e = np.exp(score - score.max())
    den = np.zeros(B_SEGS)
    np.add.at(den, batch, e)
    num = np.zeros((B_SEGS, h.shape[1]))
    np.add.at(num, batch, e[:, None] * h.astype(np.float64))
    den = np.where(den > 0, den, 1.0)
    return (num / den[:, None]).astype(np.float32)


def kernel(h, batch, W, b):
    global LAST_RESULTS
    w_vec = np.asarray(W, dtype=np.float32).reshape(-1)
    if np.min(np.abs(w_vec)) < 1e-20:
        # hw-space accumulation cannot be unscaled for (near-)zero weights
        return _np_fallback(h, batch, W, b)

    from concourse.bass_utils import run_bass_kernel_spmd

    in_maps, t_w, n_windows, segc = _prep(h, batch, W, b)
    key = (t_w, n_windows, WIN, GRP, HDMA_SPLIT, HB_BUFS, TREE_STOP)
    if key not in _CACHE:
        _CACHE[key] = _build_program(t_w, n_windows, N_CORES)
    nc = _CACHE[key]

    res = run_bass_kernel_spmd(nc, in_maps, list(range(N_CORES)), trace=False)
    LAST_RESULTS = res
    return _finish([res.results[c]["out"] for c in range(N_CORES)], W, segc)
